# revision 29
# baseline (speedup 1.0000x reference)
"""Trainium2 Bass kernel for nn_AttentionMHA: 8-way tensor-parallel over heads.

Full attention prefill: B=1, S=2048, D=4096, H=32 Q-heads, KVH=8 KV-heads,
HD=128, causal (input_pos = arange(S)).

Per-core sharding (core c of 8): Q heads 4c..4c+3, KV head c, wo columns
512c..512(c+1).

Schedule (one PE stream, dependency-staggered):
    qkvproc(0), qkvproc(1), attn(0)+AG0, qkvproc(2), attn(1)+AG1,
    qkvproc(3), attn(2)+AG2, wo(0), attn(3)+AG3, wo(1), wo(2), wo(3)
so every 4MB per-block AllGather has >=100us of PE work between its
trigger and its consumer, and the PE never idles on a collective.

Inside qkvproc(j) the K/V projections go first (pidx0) and each RoPE/
RMSNorm chain is emitted one pidx group after its projection completes, so
the cross-engine chains (Act copy/square/sqrt -> DVE recip/rope -> Pool
broadcast) always run a full pidx group (~13us) ahead of the PE
instructions that consume them.  The five Act Sqrt ops per block are
batched back-to-back so the activation table swaps exp<->sqrt only twice
per block.

Inside attn(j) the score matmuls run two key-chunks ahead of the
attn-weight (exp) consumers; the softmax denominator accumulates on the
DVE (exs += ex per chunk, f32r) and is reduced over partitions with a
single f32r 1x512 PE matmul per head, replacing a full per-chunk PE
matmul chain.  Diagonal key chunks only cover the queries that can
attend (masked columns are exp(-30000)=0 anyway).

DMA placement: x blocks alternate the sync and gpsimd queues so the next
block's tiles prefetch while the previous block's queue drains; the very
first x/wk/wv chunks are split so the PE starts ~14us earlier; wq rides
the (startup-idle) Act queue.  Act-queue DMA triggers are avoided during
attention since they fire late behind the exp stream.
"""
import os
import sys

sys.path.insert(0, "/opt/trn_rl_repo")

import numpy as np
import ml_dtypes

import concourse.bass as bass
import concourse.bass_isa as bass_isa
import concourse.tile as tile
from concourse import bacc, mybir

f32 = mybir.dt.float32
f32r = mybir.dt.float32r
bf16 = mybir.dt.bfloat16
AF = mybir.ActivationFunctionType
ALU = mybir.AluOpType

B, S, D = 1, 2048, 4096
H, KVH, HD = 32, 8, 128
NH = 4            # q heads per core
TB = 512          # token block
NT = S // TB      # 4 token blocks
KC = D // 128     # 32 contraction chunks
NKT = S // 128    # 16 key chunks
EPS = 1e-5
SCALE = 1.0 / np.sqrt(HD)
NEG = -30000.0
N_CORES = 8

DT_BIG_NAME = os.environ.get("ATTN_DT", "bf16")

SWAP_MASK = list(range(16, 32)) + list(range(0, 16))


def build_nc(dt_name=None):
    dt_name = dt_name or DT_BIG_NAME
    dtb = bf16 if dt_name == "bf16" else f32r
    dram_big = bf16 if dt_name == "bf16" else f32

    nc = bacc.Bacc("TRN2", target_bir_lowering=False, debug=False,
                   num_devices=N_CORES)

    XT = nc.dram_tensor("XT", [D, S], dram_big, kind="ExternalInput")
    WQ = nc.dram_tensor("WQ", [D, NH * HD], dram_big, kind="ExternalInput")
    WK = nc.dram_tensor("WK", [D, HD], dram_big, kind="ExternalInput")
    WV = nc.dram_tensor("WV", [D, HD], dram_big, kind="ExternalInput")
    WO = nc.dram_tensor("WO", [D, 512], dram_big, kind="ExternalInput")
    CC = nc.dram_tensor("CC", [HD, S], f32, kind="ExternalInput")
    SSI = nc.dram_tensor("SSI", [HD, S], f32, kind="ExternalInput")
    MASK = nc.dram_tensor("MASK", [128, 4 * TB], f32, kind="ExternalInput")
    IDM = nc.dram_tensor("IDM", [128, 128], f32, kind="ExternalInput")
    WQKC = nc.dram_tensor("WQKC", [128, 1], f32, kind="ExternalInput")
    ONESC = nc.dram_tensor("ONESC", [128, 1], f32, kind="ExternalInput")
    OUT = nc.dram_tensor("OUT", [S, 512], f32, kind="ExternalOutput")

    def big_view(t):
        ap = t.ap()
        return ap if dtb is bf16 else ap.bitcast(f32r)

    with tile.TileContext(nc) as tc, \
         nc.allow_low_precision(reason="intentional bf16/f32r operand rounding"):
        from contextlib import ExitStack
        with tc.tile_pool(name="dram", bufs=1, space="DRAM") as dram:
            y_ag = [dram.tile([NH * HD, TB], dram_big, name=f"yag{j}")
                    for j in range(NT)]
            y_full = [dram.tile([H * HD, TB], dram_big, addr_space="Shared",
                                name=f"yfull{j}") for j in range(NT)]
            warm_in = dram.tile([128, 8], dram_big, name="warmin")
            warm_out = dram.tile([N_CORES * 128, 8], dram_big,
                                 addr_space="Shared", name="warmout")
            ctx = ExitStack()
            with ctx:
                const = ctx.enter_context(tc.tile_pool(name="const", bufs=1))
                wqpool = ctx.enter_context(tc.tile_pool(name="wqpool", bufs=1))
                wopool = ctx.enter_context(tc.tile_pool(name="wopool", bufs=1))
                xtp = ctx.enter_context(tc.tile_pool(name="xtp", bufs=3 if dtb is bf16 else 4))
                qfp = ctx.enter_context(tc.tile_pool(name="qfp", bufs=8))
                resid = ctx.enter_context(tc.tile_pool(name="resid", bufs=1))
                tmp = ctx.enter_context(tc.tile_pool(name="tmp", bufs=2))
                smalls = ctx.enter_context(tc.tile_pool(name="smalls", bufs=1))
                expp = ctx.enter_context(tc.tile_pool(name="expp", bufs=3))
                exsp = ctx.enter_context(tc.tile_pool(name="exsp", bufs=2))
                ystp = ctx.enter_context(tc.tile_pool(name="ystp", bufs=2))
                ytp = ctx.enter_context(tc.tile_pool(name="ytp", bufs=3))
                outp = ctx.enter_context(tc.tile_pool(name="outp", bufs=2))
                # PSUM budget (8 banks): proj pa(2)+pb(1) + scores(2) + y(2)
                # + denom(1); rsp / v-transpose outputs borrow "sc" slots.
                proj = ctx.enter_context(
                    tc.tile_pool(name="proj", bufs=2, space="PSUM"))
                scoresp = ctx.enter_context(
                    tc.tile_pool(name="scoresp", bufs=2, space="PSUM"))
                ypp = ctx.enter_context(
                    tc.tile_pool(name="ypp", bufs=2, space="PSUM"))
                dpp = ctx.enter_context(
                    tc.tile_pool(name="dpp", bufs=1, space="PSUM"))

                # ---- CC warm-up: tiny AllGather to absorb the first-
                # collective cost (ring setup + inter-core start skew)
                # while the initial DMAs stream.
                warm_t = const.tile([128, 8], dram_big)
                nc.gpsimd.memset(warm_t[:], 0.0)
                nc.gpsimd.dma_start(warm_in[:], warm_t[:])
                nc.gpsimd.collective_compute(
                    "AllGather", ALU.bypass,
                    replica_groups=[list(range(N_CORES))],
                    ins=[warm_in[:]], outs=[warm_out[:]])

                # ---- constants ----
                cc_t = const.tile([HD, S], bf16)
                ss_t = const.tile([HD, S], bf16)
                mask_t = const.tile([128, 4 * TB], bf16)
                id_t = const.tile([128, 128], f32)
                nc.sync.dma_start(id_t[:], IDM.ap())
                wqkc_t = const.tile([128, 1], f32)
                nc.sync.dma_start(wqkc_t[:], WQKC.ap())
                onesc_t = const.tile([128, 1], f32r)
                nc.sync.dma_start(onesc_t[:], ONESC.ap().bitcast(f32r))
                eps_t = const.tile([1, 1], f32)
                nc.vector.memset(eps_t[:], EPS)

                wq_t = wqpool.tile([128, KC, NH * HD], dtb)
                wq_src3 = big_view(WQ).rearrange("(k p) n -> p k n", p=128)
                wo_t = wopool.tile([128, KC, 512], dtb)
                wkr_t = const.tile([128, KC, HD], dtb)
                wvr_t = const.tile([128, KC, HD], dtb)

                def first_block_loads():
                    """Spread block-0 loads over all three DMA queues
                    (~75GB/s each cold) in consumption order.  Block 0
                    projects K/V first, so the tiny wk/wv go first on the
                    gpsimd queue while the 4MB of wq streams on scalar."""
                    xta_a = xtp.tile([128, 16, TB], dtb, tag="xta")
                    xta_b = xtp.tile([128, 16, TB], dtb, tag="xta")
                    # first chunk split in two so the K/V chains start early
                    nc.sync.dma_start(xta_a[:, 0:4, :],
                                      xt_srcp[:, 0:4, 0:TB])
                    nc.sync.dma_start(xta_a[:, 4:8, :],
                                      xt_srcp[:, 4:8, 0:TB])
                    for g in range(1, 4):
                        half = xta_a if g < 2 else xta_b
                        nc.sync.dma_start(
                            half[:, (g % 2) * 8:(g % 2 + 1) * 8, :],
                            xt_srcp[:, g * 8:(g + 1) * 8, 0:TB])
                    wk_src = big_view(WK).rearrange("(k p) n -> p k n", p=128)
                    wv_src = big_view(WV).rearrange("(k p) n -> p k n", p=128)
                    nc.gpsimd.dma_start(wkr_t[:, 0:8, :], wk_src[:, 0:8, :])
                    nc.gpsimd.dma_start(wvr_t[:, 0:8, :], wv_src[:, 0:8, :])
                    nc.gpsimd.dma_start(wkr_t[:, 8:KC, :], wk_src[:, 8:KC, :])
                    nc.gpsimd.dma_start(wvr_t[:, 8:KC, :], wv_src[:, 8:KC, :])
                    for g in range(4):
                        nc.scalar.dma_start(wq_t[:, g * 8:(g + 1) * 8, :],
                                            wq_src3[:, g * 8:(g + 1) * 8, :])
                    nc.gpsimd.dma_start(cc_t[:], CC.ap())
                    nc.gpsimd.dma_start(ss_t[:], SSI.ap())
                    nc.gpsimd.dma_start(mask_t[:], MASK.ap())
                    return xta_a, xta_b

                kfin = resid.tile([128, S], dtb)
                vnat = resid.tile([128, NKT * 128], dtb)

                xt_srcp = big_view(XT).rearrange("(k p) t -> p k t", p=128)

                def part1(raw_psum, j):
                    """Act copy + square + RoPE combine (everything that
                    doesn't need the per-token rsqrt)."""
                    qs = tmp.tile([128, TB], f32, tag="qs")
                    nc.scalar.copy(qs[:], raw_psum[:])
                    sq = tmp.tile([128, TB], f32r, tag="sq", bufs=2)
                    nc.scalar.square(sq[:], raw_psum[:])
                    tsw = tmp.tile([128, TB], f32, tag="tsw", bufs=1)
                    nc.vector.stream_shuffle(tsw[:], qs[:], SWAP_MASK)
                    t1 = tmp.tile([128, TB], f32, tag="t1", bufs=2)
                    nc.vector.tensor_tensor(
                        t1[:], qs[:], cc_t[:, j * TB:(j + 1) * TB], ALU.mult)
                    t2 = tmp.tile([128, TB], f32, tag="t2", bufs=1)
                    nc.vector.tensor_tensor(
                        t2[:], tsw[:], ss_t[:, j * TB:(j + 1) * TB], ALU.mult)
                    nc.vector.tensor_tensor(t1[:], t1[:], t2[:], ALU.add)
                    return {"sq": sq, "t1": t1}

                def rspfin(st, j, is_k):
                    """rsp matmul + sqrt + recip + broadcast + final
                    normalize multiply.  Copy/square/sqrt share an act
                    table, so no table reloads inside qkvproc."""
                    rsp = scoresp.tile([128, TB], f32, tag="sc")
                    nc.tensor.matmul(rsp[0:1, :], onesc_t[:], st["sq"][:],
                                     start=True, stop=True)
                    srt = smalls.tile([1, TB], f32, tag="srt")
                    nc.scalar.activation(srt[:], rsp[0:1, :], AF.Sqrt,
                                         bias=eps_t[:], scale=1.0 / HD)
                    rr = smalls.tile([1, TB], f32, tag="rr")
                    nc.vector.reciprocal_approx_fast(rr[:], srt[:])
                    bcb = tmp.tile([128, TB], f32, tag="bcb", bufs=2)
                    nc.gpsimd.partition_broadcast(bcb[:], rr[:])
                    if is_k:
                        nc.vector.scalar_tensor_tensor(
                            kfin[:, j * TB:(j + 1) * TB], st["t1"][:],
                            wqkc_t[:], bcb[:], ALU.mult, ALU.mult)
                        return None
                    qf = qfp.tile([128, TB], dtb, tag="qf")
                    nc.vector.tensor_tensor(qf[:], st["t1"][:], bcb[:],
                                            ALU.mult)
                    return qf

                def load_xta(j):
                    t0, t1 = j * TB, (j + 1) * TB
                    xta_a = xtp.tile([128, 16, TB], dtb, tag="xta")
                    xta_b = xtp.tile([128, 16, TB], dtb, tag="xta")
                    # alternate queues per block: keeps x streaming ahead
                    # even while one queue is busy with the previous block.
                    # Block 3 splits across both: by then sync is free but
                    # gpsimd is backed up with wo weights + AG staging.
                    for g in range(4):
                        if j == 3:
                            eng = nc.sync if g % 2 == 0 else nc.gpsimd
                        else:
                            eng = nc.sync if j % 2 == 0 else nc.gpsimd
                        half = xta_a if g < 2 else xta_b
                        eng.dma_start(
                            half[:, (g % 2) * 8:(g % 2 + 1) * 8, :],
                            xt_srcp[:, g * 8:(g + 1) * 8, t0:t1])
                    return xta_a, xta_b

                def emit_qkvproc(j, xta_pre=None, ktail_prev=None,
                                 kv_first=False):
                    """Projections in three psum group-pairs with each
                    RoPE/RMSNorm chain finished one pidx group after its
                    projection completes.  Normally (q0,q1),(q2,q3),(K,V)
                    and the K/V tail (rsp(k) + transposes) is returned as a
                    closure emitted at the start of the NEXT block.  With
                    kv_first (block 0) the order is (K,V),(q0,q1),(q2,q3)
                    so the PE can start on the tiny wk/wv while the 4MB wq
                    still streams in."""
                    t0, t1 = j * TB, (j + 1) * TB
                    xta_a, xta_b = xta_pre or load_xta(j)
                    if ktail_prev is not None:
                        ktail_prev()
                    st = {}
                    q_tiles = []
                    vts = [None]
                    kv_pidx = 0 if kv_first else 2

                    def emit_kv_tail():
                        rspfin(st["k"], j, True)
                        for ci in range(4):
                            pt = scoresp.tile([128, TB], f32, tag="sc")
                            nc.tensor.transpose(
                                pt[:, 0:128],
                                vts[0][:, ci * 128:(ci + 1) * 128], id_t[:])
                            nc.vector.tensor_copy(
                                vnat[:, (4 * j + ci) * 128:
                                     (4 * j + ci + 1) * 128],
                                pt[:, 0:128])

                    for pidx in range(3):
                        pa = proj.tile([128, TB], f32, tag="pa")
                        pb = proj.tile([128, TB], f32, tag="pb", bufs=1)
                        for g in range(KC // 8):
                            half = xta_a if g < 2 else xta_b
                            xg = half[:, (g % 2) * 8:(g % 2 + 1) * 8, :]
                            for kk in range(8):
                                k = g * 8 + kk
                                sta, sp = (k == 0), (k == KC - 1)
                                if pidx == kv_pidx:
                                    nc.tensor.matmul(
                                        pa[:], wkr_t[:, k, :], xg[:, kk, :],
                                        start=sta, stop=sp)
                                    nc.tensor.matmul(
                                        pb[:], wvr_t[:, k, :], xg[:, kk, :],
                                        start=sta, stop=sp)
                                else:
                                    h0 = 2 * (pidx - 1 if kv_first else pidx)
                                    nc.tensor.matmul(
                                        pa[:],
                                        wq_t[:, k, h0 * 128:(h0 + 1) * 128],
                                        xg[:, kk, :], start=sta, stop=sp)
                                    nc.tensor.matmul(
                                        pb[:],
                                        wq_t[:, k,
                                             (h0 + 1) * 128:(h0 + 2) * 128],
                                        xg[:, kk, :], start=sta, stop=sp)
                        # post-group chain emission
                        if pidx == kv_pidx:
                            st["k"] = part1(pa, j)
                            vts[0] = tmp.tile([128, TB], f32, tag="vts",
                                              bufs=1, name="vt_s")
                            nc.vector.tensor_copy(vts[0][:], pb[:])
                        elif kv_first and pidx == 1:
                            emit_kv_tail()
                            st["q0"] = part1(pa, j)
                            st["q1"] = part1(pb, j)
                        elif kv_first and pidx == 2:
                            q_tiles.append(rspfin(st["q0"], j, False))
                            q_tiles.append(rspfin(st["q1"], j, False))
                            st["q2"] = part1(pa, j)
                            st["q3"] = part1(pb, j)
                        elif pidx == 0:
                            st["q0"] = part1(pa, j)
                            st["q1"] = part1(pb, j)
                        else:
                            q_tiles.append(rspfin(st["q0"], j, False))
                            q_tiles.append(rspfin(st["q1"], j, False))
                            st["q2"] = part1(pa, j)
                            st["q3"] = part1(pb, j)
                    q_tiles.append(rspfin(st["q2"], j, False))
                    q_tiles.append(rspfin(st["q3"], j, False))
                    if kv_first:
                        return q_tiles, None
                    return q_tiles, emit_kv_tail

                def emit_attention(j, q_tiles):
                    nchunks = 4 * (j + 1)
                    pend = [None]  # deferred per-head normalize tail

                    def chunk_off(c):
                        ci = c - 4 * j
                        qlo = max(0, ci * 128) if ci > 0 else 0
                        return ci, qlo

                    def emit_scores(h, c):
                        ci, off = chunk_off(c)
                        sc = scoresp.tile([128, TB], f32, tag="sc")
                        nc.tensor.matmul(
                            sc[:, off:TB], kfin[:, c * 128:(c + 1) * 128],
                            q_tiles[h][:, off:TB], start=True, stop=True)
                        if ci >= 0:
                            nc.vector.tensor_tensor(
                                sc[:, off:TB], sc[:, off:TB],
                                mask_t[:, ci * TB + off:(ci + 1) * TB],
                                ALU.add)
                        ex = expp.tile([128, TB], dtb, tag="ex")
                        nc.scalar.activation(ex[:, off:TB], sc[:, off:TB],
                                             AF.Exp, scale=SCALE)
                        return ex, off

                    def emit_tail(h, yp, exs):
                        # denominator reduce + reciprocal + broadcast +
                        # normalize + stage for AllGather.  The f32r reduce
                        # is issued here, one head late, so it never waits
                        # on the DVE exs chain (which is still draining at
                        # the owning head's end).
                        dps = dpp.tile([1, TB], f32, tag="dp")
                        nc.tensor.matmul(dps[:], onesc_t[:], exs[:],
                                         start=True, stop=True)
                        drec = smalls.tile([1, TB], f32, tag="drec")
                        nc.vector.reciprocal_approx_fast(drec[:],
                                                         dps[0:1, :])
                        dbc = tmp.tile([128, TB], f32, tag="dbc", bufs=1)
                        nc.gpsimd.partition_broadcast(dbc[:], drec[:])
                        yst = ystp.tile([128, TB], dram_big, tag="yst")
                        nc.vector.tensor_tensor(yst[:], yp[:], dbc[:],
                                                ALU.mult)
                        # stage on the gpsimd queue: keeps the sync queue
                        # free for bulk x/y loads (no compute-gated DMAs
                        # blocking them) and naturally precedes the AG
                        # trigger on the same queue
                        nc.gpsimd.dma_start(y_ag[j][h * HD:(h + 1) * HD, :],
                                            yst[:])

                    for h in range(NH):
                        yp = ypp.tile([128, TB], f32, tag="yp")
                        exs = exsp.tile([128, TB], f32r, tag="exs")
                        sq = [emit_scores(h, 0), emit_scores(h, 1)]
                        for c in range(nchunks):
                            if c + 2 < nchunks:
                                sq.append(emit_scores(h, c + 2))
                            if c == 1 and pend[0] is not None:
                                emit_tail(*pend[0])
                                pend[0] = None
                            ex, off = sq[c]
                            nc.tensor.matmul(
                                yp[:, off:TB],
                                vnat[:, c * 128:(c + 1) * 128], ex[:, off:TB],
                                start=(c == 0), stop=(c == nchunks - 1))
                            # denominator: accumulate exp sums on the DVE
                            # (frees one full PE matmul chain); reduced over
                            # partitions once per head below.
                            if c == 0:
                                nc.vector.tensor_copy(exs[:, off:TB],
                                                      ex[:, off:TB])
                            else:
                                nc.vector.tensor_tensor(
                                    exs[:, off:TB], exs[:, off:TB],
                                    ex[:, off:TB], ALU.add)
                        pend[0] = (h, yp, exs)
                    emit_tail(*pend[0])
                    nc.gpsimd.collective_compute(
                        "AllGather", ALU.bypass,
                        replica_groups=[list(range(N_CORES))],
                        ins=[y_ag[j][:]], outs=[y_full[j][:]])

                def emit_wo_tile(j, ti):
                    toff = ti * 128
                    srcd = y_full[j][:]
                    if dtb is f32r:
                        srcd = srcd.bitcast(f32r)
                    srcd = srcd.rearrange("(k p) t -> p k t", p=128)
                    # accumulate on the proj pa ring (idle after qkvproc):
                    # keeps the ypp ring free for the interleaved attn(3)
                    po = proj.tile([128, 512], f32, tag="pa")
                    # y reads on gpsimd/sync only (both idle here); OUT
                    # writes get the scalar queue so neither competes
                    engs = [nc.gpsimd, nc.sync, nc.gpsimd, nc.sync]
                    for g in range(4):
                        yt = ytp.tile([128, 8, 128], dtb, tag="yt")
                        engs[g].dma_start(
                            yt[:], srcd[:, g * 8:(g + 1) * 8,
                                        toff:toff + 128])
                        for kk in range(8):
                            k = g * 8 + kk
                            nc.tensor.matmul(po[:], yt[:, kk, :],
                                             wo_t[:, k, :],
                                             start=(k == 0),
                                             stop=(k == KC - 1))
                    ot = outp.tile([128, 512], f32, tag="ot")
                    nc.vector.tensor_copy(ot[:], po[:])
                    t = 4 * j + ti
                    # wo(0) precedes attn(3) in the Act stream: keep its OUT
                    # triggers off the scalar queue so the exps start on time
                    eng = nc.sync if j == 0 else nc.scalar
                    eng.dma_start(OUT.ap()[t * 128:(t + 1) * 128, :], ot[:])

                def emit_wo(j):
                    for ti in range(4):
                        emit_wo_tile(j, ti)

                # ---- schedule ----
                xta0 = first_block_loads()
                q0_t, _ = emit_qkvproc(0, xta_pre=xta0, kv_first=True)
                wo_src3 = big_view(WO).rearrange("(k p) n -> p k n", p=128)
                for g in range(4):
                    eng = nc.gpsimd if g % 2 == 0 else nc.scalar
                    eng.dma_start(wo_t[:, g * 8:(g + 1) * 8, :],
                                  wo_src3[:, g * 8:(g + 1) * 8, :])
                q1_t, kt1 = emit_qkvproc(1)
                emit_attention(0, q0_t)
                q2_t, kt2 = emit_qkvproc(2, ktail_prev=kt1)
                emit_attention(1, q1_t)
                q3_t, kt3 = emit_qkvproc(3, ktail_prev=kt2)
                kt3()
                emit_attention(2, q2_t)
                emit_wo(0)
                emit_attention(3, q3_t)
                emit_wo(1)
                emit_wo(2)
                emit_wo(3)

    nc.compile()
    return nc


_PERM = None


def _perm():
    """Within-head permutation: quadrant q holds pairs 16q..16q+15 as
    16 real rows then 16 imag rows (stream_shuffle swaps within quadrants)."""
    global _PERM
    if _PERM is None:
        p = np.zeros(HD, dtype=np.int64)
        for q in range(4):
            for jj in range(16):
                p[32 * q + jj] = 2 * (16 * q + jj)
                p[32 * q + 16 + jj] = 2 * (16 * q + jj) + 1
        _PERM = p
    return _PERM


def make_inputs(x, freqs_cos, freqs_sin, wq, wk, wv, wo, q_norm_w, k_norm_w,
                dt_name=None):
    dt_name = dt_name or DT_BIG_NAME
    np_big = ml_dtypes.bfloat16 if dt_name == "bf16" else np.float32
    perm = _perm()
    xT = np.ascontiguousarray(x.reshape(S, D).T).astype(np_big)
    cosT = np.ascontiguousarray(freqs_cos.T)  # [64, S]
    sinT = np.ascontiguousarray(freqs_sin.T)
    cc = np.empty((HD, S), dtype=np.float32)
    ssg = np.empty((HD, S), dtype=np.float32)
    for q in range(4):
        cc[32 * q:32 * q + 16] = cosT[16 * q:16 * q + 16]
        cc[32 * q + 16:32 * q + 32] = cosT[16 * q:16 * q + 16]
        ssg[32 * q:32 * q + 16] = -sinT[16 * q:16 * q + 16]
        ssg[32 * q + 16:32 * q + 32] = sinT[16 * q:16 * q + 16]
    mask = np.empty((128, 4 * TB), dtype=np.float32)
    qt = np.arange(TB)
    for ci in range(4):
        kt = 128 * ci + np.arange(128)
        mask[:, ci * TB:(ci + 1) * TB] = np.where(
            kt[:, None] <= qt[None, :], 0.0, NEG).astype(np.float32)
    wqk = (q_norm_w * k_norm_w)[perm].reshape(HD, 1).astype(np.float32)
    common = dict(
        XT=xT, CC=cc, SSI=ssg, MASK=mask,
        IDM=np.eye(128, dtype=np.float32), WQKC=wqk,
        ONESC=np.ones((128, 1), dtype=np.float32),
    )
    in_maps = []
    for c in range(N_CORES):
        wq_c = wq[:, c * NH * HD:(c + 1) * NH * HD].reshape(D, NH, HD)
        wq_c = np.ascontiguousarray(wq_c[:, :, perm].reshape(D, NH * HD))
        wk_c = np.ascontiguousarray(wk[:, c * HD:(c + 1) * HD][:, perm])
        wv_c = np.ascontiguousarray(wv[:, c * HD:(c + 1) * HD])
        wo_c = np.ascontiguousarray(wo[:, c * 512:(c + 1) * 512])
        in_maps.append(dict(
            common, WQ=wq_c.astype(np_big), WK=wk_c.astype(np_big),
            WV=wv_c.astype(np_big), WO=wo_c.astype(np_big)))
    return in_maps


_NC = None


def get_nc():
    global _NC
    if _NC is None:
        _NC = build_nc()
    return _NC


def kernel(x, freqs_cos, freqs_sin, input_pos, wq, wk, wv, wo,
           q_norm_w, k_norm_w, k_cache, v_cache):
    from concourse.bass_utils import run_bass_kernel_spmd
    nc = get_nc()
    in_maps = make_inputs(np.asarray(x), np.asarray(freqs_cos),
                          np.asarray(freqs_sin), np.asarray(wq),
                          np.asarray(wk), np.asarray(wv), np.asarray(wo),
                          np.asarray(q_norm_w), np.asarray(k_norm_w))
    res = run_bass_kernel_spmd(nc, in_maps, core_ids=list(range(N_CORES)))
    out = np.concatenate([res.results[c]["OUT"] for c in range(N_CORES)],
                         axis=1)
    return out.reshape(B, S, D).astype(np.float32)

